# revision 9
# baseline (speedup 1.0000x reference)
"""Trainium2 Bass kernel for nn_CrossInferenceBlock (bilinear cross attention).

Computation (T=256, S=256, F=1024, A=256):
    theta = (x @ a_w + a_b).reshape(T, S, A)
    phi   = (x @ b_w + b_b).reshape(T, S, A)
    feats = (x @ g_w + g_b).reshape(T, S, F)
    attn  = einsum("tsa,tra->tsr", theta, phi)
    out   = einsum("tsr,trf->tsf", attn, feats) / (S + T)

Sharding: data-parallel over t — each of the 8 cores takes 32 contiguous
t-slices; the Linear weights are replicated.

v3 changes over the 432us baseline (trace-driven):
  - DMA_DIRECT2D occupies an engine queue ~650ns per instruction, so all
    input loads through one queue serialize their ISSUE. Inputs now issue
    from three queues in parallel: x slices on GpSimd, g_w slices and the
    small weights on Scalar, output stores alone on Sync.
  - Group 0 computes feats(t0)/feats(t1) FIRST, kt-outer, consuming per-kt
    x/g_w slices as they arrive (~9.5us) instead of waiting ~17.6us for
    the whole transfer; theta/phi follow when a_w/b_w land.
  - A burst of dummy matmuls on a memset tile warms the PE_HAM clock gate
    (2.4 GHz) before the first real matmul.
  - theta/phi matmuls process a PAIR of t-slices per instruction (N=512).
  - PSUM evictions alternate Vector/Scalar so eviction chains never gate
    the PE; output is stored fp16 (host upcasts), 2KB DMA rows.

Matmuls run in fp16 (fp32 PSUM accumulation). fp8 was evaluated and
rejected: TRN e4m3 gives ~3.5% rel error on this data (tolerance 2e-2)
and DoubleRow is only ~1.5x, so no precision/speed tradeoff works.
"""

import numpy as np

import concourse.bass as bass
import concourse.bacc as bacc
import concourse.tile as tile
from concourse import mybir
from concourse.bass_utils import run_bass_kernel_spmd

T, S, F, A = 256, 256, 1024, 256
N_CORES = 8
T_LOC = T // N_CORES          # 32 t-slices per core
P = 128
KT = F // P                   # 8 contraction tiles over F
MT_A = A // P                 # 2 output tiles over A
MT_S = S // P                 # 2 tiles over s (rows of one t-slice)
NF = 512                      # matmul free-dim chunk for F-wide outputs
NC_F = F // NF                # 2 chunks
TG = 4                        # t-slices fetched per input DMA group
NG = T_LOC // TG              # 8 DMA groups per core
N_WARM = 24                   # dummy warm-up matmuls (~2.6us cold)
OUT_SCALE = 1.0 / (S + T)

F16 = mybir.dt.float16
F32 = mybir.dt.float32

_COMPILED = None


def _build():
    nc = bacc.Bacc("TRN2", target_bir_lowering=False, debug=False)

    # All inputs are host-prearranged so every DMA reads per-partition
    # CONTIGUOUS runs, keeping HBM transfers at full rate.
    # x: (NG, P, KT, TG, S) with t = g*TG + ti, f = kt*P + p.
    x_d = nc.dram_tensor("x", [NG, P, KT, TG, S], F16, kind="ExternalInput")
    aw_d = nc.dram_tensor("aw", [P, KT, MT_A, P], F16, kind="ExternalInput")
    bw_d = nc.dram_tensor("bw", [P, KT, MT_A, P], F16, kind="ExternalInput")
    gw_d = nc.dram_tensor("gw", [P, KT, F], F16, kind="ExternalInput")
    ab_d = nc.dram_tensor("ab", [A], F32, kind="ExternalInput")
    bb_d = nc.dram_tensor("bb", [A], F32, kind="ExternalInput")
    gb_d = nc.dram_tensor("gb", [F], F16, kind="ExternalInput")
    out_d = nc.dram_tensor("out", [T_LOC, S, F], F16, kind="ExternalOutput")

    x_ap = x_d.ap()
    aw_ap = aw_d.ap()
    bw_ap = bw_d.ap()
    gw_ap = gw_d.ap()
    ab_ap = ab_d.ap().rearrange("(mt p) -> p mt", p=P)
    bb_ap = bb_d.ap().rearrange("(mt p) -> p mt", p=P)
    out_ap = out_d.ap()

    with tile.TileContext(nc) as tc:
        with (
            tc.tile_pool(name="const", bufs=1) as const,
            tc.tile_pool(name="xin", bufs=3) as xin,
            tc.tile_pool(name="proj", bufs=3) as proj,
            tc.tile_pool(name="fsb", bufs=6) as fsb,
            tc.tile_pool(name="asb", bufs=4) as asb,
            tc.tile_pool(name="osb", bufs=6) as osb,
            tc.tile_pool(name="ps_a", bufs=4, space="PSUM") as ps_a,
            tc.tile_pool(name="ps_b", bufs=4, space="PSUM") as ps_b,
        ):
            # --- PE_HAM warm-up: dummy matmuls on a memset tile. These
            # depend only on engine init (~6.5us), not on any DMA, so the
            # clock gate reaches 8/8 before the first real matmul.
            warm_sb = const.tile([P, P], F16)
            nc.vector.memset(warm_sb[:], 0.0)
            warm_ps = ps_a.tile([P, NF], F32, tag="ps")
            for _ in range(N_WARM):
                nc.tensor.matmul(
                    warm_ps[:, :P], lhsT=warm_sb[:], rhs=warm_sb[:],
                    start=True, stop=True,
                )

            # --- Startup DMAs. x slices issue on the GpSimd queue while
            # gw slices + small weights issue on Scalar, so neither
            # serializes behind the other's ~650ns-per-DMA issue cost.
            # Stores get the Sync queue to themselves.
            xt0 = xin.tile([P, KT, TG, S], F16, tag="xt")
            for kt in range(KT):
                nc.gpsimd.dma_start(out=xt0[:, kt], in_=x_ap[0, :, kt])

            gw_sb = const.tile([P, KT, F], F16)
            for kt in range(KT):
                nc.scalar.dma_start(out=gw_sb[:, kt], in_=gw_ap[:, kt])
            ab_sb = const.tile([P, MT_A], F32)
            nc.scalar.dma_start(out=ab_sb[:], in_=ab_ap)
            bb_sb = const.tile([P, MT_A], F32)
            nc.scalar.dma_start(out=bb_sb[:], in_=bb_ap)
            aw_sb = const.tile([P, KT, MT_A, P], F16)
            nc.scalar.dma_start(out=aw_sb[:], in_=aw_ap)
            bw_sb = const.tile([P, KT, MT_A, P], F16)
            nc.scalar.dma_start(out=bw_sb[:], in_=bw_ap)
            gbb_sb = const.tile([P, F], F16)
            gb_bcast = bass.AP(
                tensor=gb_d.ap().tensor,
                offset=gb_d.ap().offset,
                ap=[[0, P], [1, F]],
            )
            nc.scalar.dma_start(out=gbb_sb[:], in_=gb_bcast)

            def emit_theta_phi(xt, half):
                """theta/phi for t-slice pair (2*half, 2*half+1): N=512."""
                thetaT = proj.tile([P, MT_A, 2, S], F16, tag="thetaT")
                phiT = proj.tile([P, MT_A, 2, S], F16, tag="phiT")
                pss = [
                    [
                        ps_a.tile([P, NF], F32, tag="ps", name=f"ps_p{pj}m{mt}")
                        for mt in range(MT_A)
                    ]
                    for pj in range(2)
                ]
                for kt in range(KT):
                    for pj, w_sb in enumerate((aw_sb, bw_sb)):
                        for mt in range(MT_A):
                            nc.tensor.matmul(
                                pss[pj][mt][:],
                                lhsT=w_sb[:, kt, mt, :],
                                rhs=xt[:, kt, 2 * half : 2 * half + 2, :],
                                start=(kt == 0),
                                stop=(kt == KT - 1),
                            )
                for pj, (dst, b_sb) in enumerate(
                    ((thetaT, ab_sb), (phiT, bb_sb))
                ):
                    for mt in range(MT_A):
                        nc.vector.tensor_scalar_add(
                            dst[:, mt], pss[pj][mt][:], b_sb[:, mt : mt + 1]
                        )
                return thetaT, phiT

            def emit_attn(thetaT, phiT, tip):
                """attnT[r, s] for one t; scale folded into the eviction,
                which is split Scalar/Vector to halve the chain."""
                attnT = asb.tile([P, MT_S, S], F16, tag="attnT")
                for rt in range(MT_S):
                    ps = ps_a.tile([P, NF], F32, tag="ps")
                    for at in range(MT_A):
                        nc.tensor.matmul(
                            ps[:, :S],
                            lhsT=phiT[:, at, tip, rt * P : (rt + 1) * P],
                            rhs=thetaT[:, at, tip, :],
                            start=(at == 0),
                            stop=(at == MT_A - 1),
                        )
                    if rt == 0:
                        nc.scalar.activation(
                            out=attnT[:, rt, :],
                            in_=ps[:, :S],
                            func=mybir.ActivationFunctionType.Copy,
                            scale=OUT_SCALE,
                        )
                    else:
                        nc.vector.tensor_scalar_mul(
                            attnT[:, rt, :], ps[:, :S], OUT_SCALE
                        )
                return attnT

            def emit_feats_kt_outer(xt, ti):
                """feats for one t, kt-outer: consumes the per-kt x/gw DMA
                slices as they arrive (startup path)."""
                psf = [
                    ps_b.tile([P, NF], F32, tag="ps", name=f"psf{i}")
                    for i in range(MT_S * NC_F)
                ]
                for kt in range(KT):
                    for mt in range(MT_S):
                        for c in range(NC_F):
                            nc.tensor.matmul(
                                psf[mt * NC_F + c][:],
                                lhsT=xt[:, kt, ti, mt * P : (mt + 1) * P],
                                rhs=gw_sb[:, kt, c * NF : (c + 1) * NF],
                                start=(kt == 0),
                                stop=(kt == KT - 1),
                            )
                feats = fsb.tile([P, MT_S, F], F16, tag="feats")
                for mt in range(MT_S):
                    for c in range(NC_F):
                        nc.vector.tensor_add(
                            feats[:, mt, c * NF : (c + 1) * NF],
                            psf[mt * NC_F + c][:],
                            gbb_sb[:, c * NF : (c + 1) * NF],
                        )
                return feats

            def emit_feats(xt, ti, pool):
                """feats[s, f] for one t (kt-inner; evictions interleave
                with the following psum group's matmuls)."""
                feats = fsb.tile([P, MT_S, F], F16, tag="feats")
                for mt in range(MT_S):
                    for c in range(NC_F):
                        ps = pool.tile([P, NF], F32, tag="ps", name="ps_f")
                        for kt in range(KT):
                            nc.tensor.matmul(
                                ps[:],
                                lhsT=xt[:, kt, ti, mt * P : (mt + 1) * P],
                                rhs=gw_sb[:, kt, c * NF : (c + 1) * NF],
                                start=(kt == 0),
                                stop=(kt == KT - 1),
                            )
                        nc.vector.tensor_add(
                            feats[:, mt, c * NF : (c + 1) * NF],
                            ps[:],
                            gbb_sb[:, c * NF : (c + 1) * NF],
                        )
                return feats

            def emit_out(t, attnT, feats):
                """out[s, f] = sum_r attnT[r, s] feats[r, f]; the two chunk
                evictions run on Vector and Scalar in parallel, then one
                fp16 store per (t, mt) with 2KB rows."""
                for mt in range(MT_S):
                    out_sb = osb.tile([P, F], F16, tag="out_sb")
                    for c in range(NC_F):
                        ps = ps_b.tile([P, NF], F32, tag="ps", name="ps_o")
                        for rt in range(MT_S):
                            nc.tensor.matmul(
                                ps[:],
                                lhsT=attnT[:, rt, mt * P : (mt + 1) * P],
                                rhs=feats[:, rt, c * NF : (c + 1) * NF],
                                start=(rt == 0),
                                stop=(rt == MT_S - 1),
                            )
                        if c == 0:
                            nc.vector.tensor_copy(
                                out_sb[:, c * NF : (c + 1) * NF], ps[:]
                            )
                        else:
                            nc.scalar.activation(
                                out=out_sb[:, c * NF : (c + 1) * NF],
                                in_=ps[:],
                                func=mybir.ActivationFunctionType.Copy,
                            )
                    nc.sync.dma_start(
                        out=out_ap[t, mt * P : (mt + 1) * P, :],
                        in_=out_sb[:],
                    )

            for g in range(NG):
                if g == 0:
                    xt = xt0
                    # Startup: feats(t0) runs kt-outer, consuming the x/gw
                    # slices as they arrive; theta/phi start once a_w/b_w
                    # land behind the gw stream.
                    f0sb = emit_feats_kt_outer(xt, 0)
                    f1sb = emit_feats(xt, 1, ps_b)
                    f2sb = emit_feats(xt, 2, ps_b)
                    f3sb = emit_feats(xt, 3, ps_b)
                    th0, ph0 = emit_theta_phi(xt, 0)
                    th1, ph1 = emit_theta_phi(xt, 1)
                    a0 = emit_attn(th0, ph0, 0)
                    a1 = emit_attn(th0, ph0, 1)
                    emit_out(0, a0, f0sb)
                    a2 = emit_attn(th1, ph1, 0)
                    emit_out(1, a1, f1sb)
                    a3 = emit_attn(th1, ph1, 1)
                    emit_out(2, a2, f2sb)
                    emit_out(3, a3, f3sb)
                else:
                    xt = xin.tile([P, KT, TG, S], F16, tag="xt")
                    nc.gpsimd.dma_start(out=xt[:], in_=x_ap[g])
                    th0, ph0 = emit_theta_phi(xt, 0)
                    th1, ph1 = emit_theta_phi(xt, 1)
                    projs = [
                        (th0, ph0, 0), (th0, ph0, 1), (th1, ph1, 0), (th1, ph1, 1)
                    ]
                    for ti, (thp, php, tip) in enumerate(projs):
                        t = g * TG + ti
                        at_t = emit_attn(thp, php, tip)
                        f_t = emit_feats(xt, ti, ps_b)
                        emit_out(t, at_t, f_t)

    nc.compile()
    return nc


def _get_compiled():
    global _COMPILED
    if _COMPILED is None:
        _COMPILED = _build()
    return _COMPILED


def _prep_inputs(inputs):
    x = np.asarray(inputs["batch_data"], dtype=np.float32)
    assert x.shape == (T * S, F), x.shape
    # (T, S, F) -> per-core (T_LOC, F, S) -> (NG, TG, KT, P, S) -> (NG, P, KT, TG, S)
    x16 = (
        x.reshape(T, S, F)
        .transpose(0, 2, 1)
        .astype(np.float16)
        .reshape(N_CORES, NG, TG, KT, P, S)
        .transpose(0, 1, 4, 3, 2, 5)
    )
    x16 = np.ascontiguousarray(x16)

    def tile_w(w, mt):  # (F, N) -> (P, KT, mt, 128)
        n = w.shape[1]
        return np.ascontiguousarray(
            w.astype(np.float16).reshape(KT, P, mt, n // mt).transpose(1, 0, 2, 3)
        )

    aw16 = tile_w(np.asarray(inputs["a_w"], np.float32), MT_A)
    bw16 = tile_w(np.asarray(inputs["b_w"], np.float32), MT_A)
    gw16 = tile_w(np.asarray(inputs["g_w"], np.float32), 1).reshape(P, KT, F)
    ab32 = np.ascontiguousarray(np.asarray(inputs["a_b"], np.float32))
    bb32 = np.ascontiguousarray(np.asarray(inputs["b_b"], np.float32))
    gb16 = np.ascontiguousarray(np.asarray(inputs["g_b"], np.float32).astype(np.float16))
    in_maps = []
    for c in range(N_CORES):
        in_maps.append(
            {
                "x": x16[c],
                "aw": aw16,
                "bw": bw16,
                "gw": gw16,
                "ab": ab32,
                "bb": bb32,
                "gb": gb16,
            }
        )
    return in_maps


def run_spmd(inputs, **kwargs):
    """Run the compiled kernel; returns (full_output, BassKernelResults)."""
    nc = _get_compiled()
    in_maps = _prep_inputs(inputs)
    res = run_bass_kernel_spmd(nc, in_maps, list(range(N_CORES)), **kwargs)
    out = np.concatenate(
        [
            np.asarray(res.results[c]["out"]).astype(np.float32)
            for c in range(N_CORES)
        ],
        axis=0,
    )
    return out, res


def kernel(**inputs) -> np.ndarray:
    out, _ = run_spmd(inputs)
    return out
